# revision 41
# baseline (speedup 1.0000x reference)
"""BitSelfAttention on 8 TRN2 NeuronCores.

Sharding: core c handles batch b = c//2 and head-group hg = c%2 (8 of 16 heads).
Each core computes its 8 heads' QKV projections + causal attention + its slice
of the o_proj GEMM, producing a partial output (transposed, [D, T], fp32).
Host: pre-quantizes BitLinear weights (ternary * gamma, exact in bf16),
pre-transposes operands into matmul-friendly layouts, and sums the two
head-group partials per batch at the end.

Device layouts (per core):
  xT   [D, T]  bf16 : x[b].T              (rhs for Q/K/V^T projections)
  wqT  [D, F]  bf16 : w_q_eff[hg-rows].T  (stationary tiles for Q^T proj)
  wkT  [D, F]  bf16
  wvT  [D, F]  bf16
  woT  [F, D]  bf16 : w_o_eff[:, hg-cols].T (stationary tiles for o_proj)
  cmask[4, 128, 512] bf16 : causal masks for the 4 diagonal offsets
  outT [D, T]  fp32 : partial output, transposed

Per head h: Q^T,K^T [dh=128, T] (dh-major), V^T transposed on the PE into
token-major V tiles. Attention computed as S^T = K^T_tile.T @ Q^T_block so
softmax rows land on the free axis; P^T = exp(S^T*scale) (ACT, PSUM->SBUF
bf16); key-tile partial row-sums accumulate in fp32 on the vector engine and
one all-ones stationary matmul per block reduces across partitions while
broadcasting the result to every partition (so normalization needs no
cross-partition broadcast); O^T = V_tile.T @ P^T accumulated over key tiles;
normalize with fast-reciprocal+multiply during PSUM eviction. o_proj consumes
O^T tiles directly as stationary operands, producing outT; its per-token-block
chains double as PE fill work zipped into the last head's attention, just as
each head's projection chains are zipped into the previous head's attention
(the attention inner loop is otherwise exp-latency-gated on the in-order PE).
"""

import math

import ml_dtypes
import numpy as np

import concourse.mybir as mybir
import concourse.tile as tile
from concourse import bacc
from concourse import bass_utils
from concourse.masks import make_identity

BF16 = mybir.dt.bfloat16
F32 = mybir.dt.float32

D_MODEL = 2048
N_HEAD = 16
D_HEAD = 128
B = 4
T_FULL = 2048
N_CORES = 8
F_LOC = D_MODEL // 2  # features per core (8 heads)


def build_bass(T=T_FULL, D=D_MODEL, F=F_LOC, debug=False):
    """Build the single-core program (SPMD across 8 cores via input data)."""
    P = 128
    KD = D // P      # contraction 128-tiles
    TT = T // P      # token 128-tiles
    TB = T // 512    # token 512-blocks
    H = F // P       # local heads
    MT = D // P      # output-dmodel 128-tiles
    KT_PER_B = 512 // P
    SCALE = 1.0 / math.sqrt(D_HEAD)

    nc = bacc.Bacc("TRN2", target_bir_lowering=False, debug=debug,
                   num_devices=N_CORES)
    xT_d = nc.dram_tensor("xT", [D, T], BF16, kind="ExternalInput").ap()
    # weights pre-tiled on host into the exact SBUF layouts (contiguous DMAs):
    #   wqT/wkT/wvT: [H, 128, KD*128] with [h, p, kd*128+f] = w_eff[h*128+f, kd*128+p]
    #   woT:         [MT, 128, H*128] with [m, p, h*128+j] = wo_eff[m*128+j, h*128+p]
    H_ = F // P
    MT_ = D // P
    KD_ = D // P
    wqT_d = nc.dram_tensor("wqT", [H_, P, KD_ * P], BF16,
                           kind="ExternalInput").ap()
    wkT_d = nc.dram_tensor("wkT", [H_, P, KD_ * P], BF16,
                           kind="ExternalInput").ap()
    wvT_d = nc.dram_tensor("wvT", [H_, P, KD_ * P], BF16,
                           kind="ExternalInput").ap()
    woT_d = nc.dram_tensor("woT", [MT_, P, H_ * P], BF16,
                           kind="ExternalInput").ap()
    cm_d = nc.dram_tensor("cmask", [4, P, 512], BF16, kind="ExternalInput").ap()
    out_d = nc.dram_tensor("outT", [D, T], F32, kind="ExternalOutput").ap()

    with tile.TileContext(nc) as tc:
        with (
            tc.tile_pool(name="big", bufs=1) as big,
            tc.tile_pool(name="work", bufs=2) as work,
            tc.tile_pool(name="psS", bufs=3, space="PSUM") as psS,
            tc.tile_pool(name="psO", bufs=2, space="PSUM") as psO,
            tc.tile_pool(name="psR", bufs=1, space="PSUM") as psR,
            tc.tile_pool(name="psP", bufs=2, space="PSUM") as psP,
        ):
            # ---- persistent inputs (head-0 weights first: first MMs need them)
            # head-0 V weights in kd-progressive chunks: the first projection
            # chain consumes kd in order, so it can start ~10us earlier than
            # waiting for one whole-tile DMA
            wvh0 = work.tile([P, KD, P], BF16, name="wvh0", tag="wvh")
            nck = KD // 4
            for g in range(4):
                nc.sync.dma_start(
                    out=wvh0[:, g * nck:(g + 1) * nck, :],
                    in_=wvT_d[0, :, g * nck * P:(g + 1) * nck * P])
            ones = big.tile([P, P], BF16, name="ones_sb", tag="ones", bufs=1)
            nc.vector.memset(ones, 1.0)
            ident = big.tile([P, P], BF16, name="ident_sb", tag="ident", bufs=1)
            make_identity(nc, ident)
            xt = []
            for kd in range(KD):
                xti = big.tile([P, T], BF16, name=f"xt{kd}", tag="xt", bufs=KD)
                nc.sync.dma_start(out=xti, in_=xT_d[kd * P:(kd + 1) * P, :])
                xt.append(xti)
            cmask = big.tile([P, 4, 512], BF16, name="cmask_sb", tag="cmask",
                             bufs=1)
            for i in range(4):
                nc.sync.dma_start(out=cmask[:, i, :], in_=cm_d[i])
            ot = [big.tile([P, T], BF16, name=f"ot{h}", tag="ot", bufs=H)
                  for h in range(H)]

            # ---- per-head pipeline with cross-head fill interleaving.
            # The attention inner loop is ACT(exp)-gated by ~40ns/iter; we
            # pump one projection matmul of the NEXT head between attention
            # iterations so the (in-order) PE always has fill work.
            def load_head_weights(h, wvh=None):
                if wvh is None:
                    wvh = work.tile([P, KD, P], BF16, name=f"wvh{h}",
                                    tag="wvh")
                    nc.sync.dma_start(out=wvh.rearrange("p kd f -> p (kd f)"),
                                      in_=wvT_d[h])
                wqh = work.tile([P, KD, P], BF16, name=f"wqh{h}", tag="wqh")
                nc.sync.dma_start(out=wqh.rearrange("p kd f -> p (kd f)"),
                                  in_=wqT_d[h])
                wkh = work.tile([P, KD, P], BF16, name=f"wkh{h}", tag="wkh")
                nc.sync.dma_start(out=wkh.rearrange("p kd f -> p (kd f)"),
                                  in_=wkT_d[h])
                return wqh, wkh, wvh

            def load_qk_weights(h):
                wqh = work.tile([P, KD, P], BF16, name=f"wqh{h}", tag="wqh")
                nc.sync.dma_start(out=wqh.rearrange("p kd f -> p (kd f)"),
                                  in_=wqT_d[h])
                wkh = work.tile([P, KD, P], BF16, name=f"wkh{h}", tag="wkh")
                nc.sync.dma_start(out=wkh.rearrange("p kd f -> p (kd f)"),
                                  in_=wkT_d[h])
                return wqh, wkh

            def alloc_head_tiles(h):
                vT = work.tile([P, T], BF16, name=f"vT{h}", tag="vT")
                vh = work.tile([P, TT, P], BF16, name=f"vh{h}", tag="vh")
                qt_ = work.tile([P, T], BF16, name=f"qt{h}", tag="qt")
                kt_ = work.tile([P, T], BF16, name=f"kt{h}", tag="kt")
                return vT, vh, qt_, kt_

            def proj_fill_gen(ws, tiles):
                """V^T then Q^T then K^T projection chains, yielding after
                every matmul so the caller can interleave them."""
                wqh, wkh, wvh = ws
                vT, vh, qt_, kt_ = tiles
                for wh, dst in ((wvh, vT), (wqh, qt_), (wkh, kt_)):
                    for tb in range(TB):
                        ts_ = slice(tb * 512, (tb + 1) * 512)
                        ps = psP.tile([P, 512], F32, name="psfill", tag="psp")
                        for kd in range(KD):
                            nc.tensor.matmul(ps, lhsT=wh[:, kd, :],
                                             rhs=xt[kd][:, ts_],
                                             start=(kd == 0),
                                             stop=(kd == KD - 1))
                            yield
                        nc.vector.tensor_copy(out=dst[:, ts_], in_=ps)

            def pump(gen, n):
                for _ in range(n):
                    try:
                        next(gen)
                    except StopIteration:
                        return False
                return True

            def pump_n(gen, n):
                c = 0
                for _ in range(n):
                    try:
                        next(gen)
                        c += 1
                    except StopIteration:
                        break
                return c

            def oproj_nb_gen(nb):
                """o_proj chains for one token block (needs all heads' ot
                columns of that block only), yielding per matmul."""
                ns = slice(nb * 512, (nb + 1) * 512)
                for m in range(MT):
                    woh = work.tile([P, H, P], BF16, name=f"woh{nb}_{m}",
                                    tag="woh", bufs=4)
                    nc.sync.dma_start(out=woh.rearrange("p h f -> p (h f)"),
                                      in_=woT_d[m])
                    yield  # let attention matmuls cover the woh DMA latency
                    ps = psP.tile([P, 512], F32, name="psout", tag="psp")
                    for hh in range(H):
                        nc.tensor.matmul(ps, lhsT=woh[:, hh, :],
                                         rhs=ot[hh][:, ns],
                                         start=(hh == 0), stop=(hh == H - 1))
                        yield
                    stg = work.tile([P, 512], F32, name="ostage", tag="ostage",
                                    bufs=4)
                    nc.vector.tensor_copy(out=stg, in_=ps)
                    nc.sync.dma_start(out=out_d[m * P:(m + 1) * P, ns],
                                      in_=stg)

            # head-0 Q/K weights and head-1 weights load after xt (the V^T
            # chains consume xt first; the Q chains run ~4 chain-times later)
            ws_list = [None] * (H + 2)
            wqh0, wkh0 = load_qk_weights(0)
            ws_list[0] = (wqh0, wkh0, wvh0)
            if H > 1:
                ws_list[1] = load_head_weights(1)
            cur_tiles = alloc_head_tiles(0)
            g0 = proj_fill_gen(ws_list[0], cur_tiles)
            while pump(g0, 1):
                pass

            fills = []

            def pump_fills(n):
                while n > 0 and fills:
                    n -= pump_n(fills[0], n)
                    if n > 0:
                        fills.pop(0)

            for h in range(H):
                vT, vh, qt_, kt_ = cur_tiles
                # prefetch weights two heads ahead so fill matmuls never
                # wait on their DMA (a blocked fill stalls the in-order PE)
                if h + 2 < H:
                    ws_list[h + 2] = load_head_weights(h + 2)
                if h + 1 < H:
                    next_tiles = alloc_head_tiles(h + 1)
                    fills.append(proj_fill_gen(ws_list[h + 1], next_tiles))
                else:
                    next_tiles = None

                def emit_transpose(kt):
                    # lives in the psS pool: psP slots are held long by
                    # in-flight interleaved fill chains
                    pst = psS.tile([P, 512], BF16, name="pst", tag="pss")
                    nc.tensor.transpose(pst[:, 0:P],
                                        vT[:, kt * P:(kt + 1) * P], ident)
                    nc.vector.tensor_copy(out=vh[:, kt, :], in_=pst[:, 0:P])

                # causal attention, S^T layout (keys on partitions).
                # Diagonal tiles (kt = 4*qb+di) only contribute to query
                # columns >= 128*di of the block; narrow S/exp/O/R to the
                # live columns. Only the first 128 columns of a (narrowed)
                # diagonal tile are triangular; the rest are fully allowed.
                for qb in range(TB):
                    nkt = KT_PER_B * (qb + 1)
                    for kt in range(KT_PER_B * qb, nkt):
                        emit_transpose(kt)
                    psO_t = psO.tile([P, 512], F32, name="psodt", tag="pso")
                    racc = work.tile([P, 512], F32, name="racc", tag="racc")
                    for kt in range(nkt):
                        di = kt - KT_PER_B * qb
                        c0 = max(di, 0) * P  # first live query column
                        w = 512 - c0
                        qs = slice(qb * 512 + c0, (qb + 1) * 512)
                        psS_t = psS.tile([P, 512], F32, name="pssc", tag="pss")
                        nc.tensor.matmul(psS_t[:, :w],
                                         lhsT=kt_[:, kt * P:(kt + 1) * P],
                                         rhs=qt_[:, qs],
                                         start=True, stop=True)
                        pt = work.tile([P, 512], BF16, name="pexp", tag="pt",
                                       bufs=6)
                        nc.scalar.activation(
                            out=pt[:, :w], in_=psS_t[:, :w],
                            func=mybir.ActivationFunctionType.Exp, scale=SCALE)
                        if di >= 0:
                            nc.vector.tensor_mul(pt[:, :P], pt[:, :P],
                                                 cmask[:, 0, :P])
                        nc.tensor.matmul(psO_t[:, c0:], lhsT=vh[:, kt, :],
                                         rhs=pt[:, :w],
                                         start=(kt == 0), stop=(kt == nkt - 1),
                                         skip_group_check=True)
                        # fp32 running key-tile sum on DVE (hidden behind the
                        # exp pacing); one ones-matmul at the end reduces
                        # across partitions and broadcasts
                        if kt == 0:
                            nc.vector.tensor_copy(out=racc, in_=pt)
                        else:
                            nc.vector.tensor_add(racc[:, c0:], racc[:, c0:],
                                                 pt[:, :w])
                        pump_fills(1 + (kt & 1))
                    raccb = work.tile([P, 512], BF16, name="raccb", tag="raccb")
                    nc.vector.tensor_copy(out=raccb, in_=racc)
                    psR_t = psR.tile([P, 512], F32, name="psrow", tag="psr")
                    nc.tensor.matmul(psR_t, lhsT=ones, rhs=raccb,
                                     start=True, stop=True)
                    rec = work.tile([P, 512], F32, name="rec", tag="rec")
                    nc.vector.reciprocal_approx_fast(out=rec, in_=psR_t)
                    nc.vector.tensor_mul(ot[h][:, qb * 512:(qb + 1) * 512],
                                         psO_t, rec)
                    if h == H - 1:
                        # this token block's ot columns are now complete for
                        # every head: its o_proj chains become fill work
                        fills.append(oproj_nb_gen(qb))
                    pump_fills(4)
                if h < H - 1:
                    # finish next head's projections before its attention
                    while fills:
                        pump_fills(64)
                cur_tiles = next_tiles
            # drain remaining o_proj work
            while fills:
                pump_fills(64)

    nc.compile()
    return nc


def _bitlinear_eff(w):
    """Forward-effective BitLinear weight: clip(round(w/gamma),-1,1)*gamma."""
    w = np.asarray(w, dtype=np.float32)
    gamma = max(np.float32(np.abs(w).mean()), np.float32(1e-5))
    q = np.clip(np.round(w / gamma), -1.0, 1.0).astype(np.float32)
    return q * gamma


def _causal_masks():
    k = np.arange(128)[:, None]
    q = np.arange(512)[None, :]
    m = np.stack([(k <= q - 128 * i) for i in range(4)]).astype(np.float32)
    return m.astype(ml_dtypes.bfloat16)


def _tile_qkv(w_shard):
    """[F, D] -> [H, 128, KD*128]: [h, p, kd*128+f] = w_shard[h*128+f, kd*128+p]."""
    Fs, Ds = w_shard.shape
    a = w_shard.reshape(Fs // 128, 128, Ds // 128, 128)  # [h, f, kd, p]
    a = a.transpose(0, 3, 2, 1).reshape(Fs // 128, 128, Ds)
    return np.ascontiguousarray(a)


def _tile_wo(wo_shard):
    """[D, F] -> [MT, 128, H*128]: [m, p, h*128+j] = wo_shard[m*128+j, h*128+p]."""
    Ds, Fs = wo_shard.shape
    a = wo_shard.reshape(Ds // 128, 128, Fs // 128, 128)  # [m, j, h, p]
    a = a.transpose(0, 3, 2, 1).reshape(Ds // 128, 128, Fs)
    return np.ascontiguousarray(a)


def _prep_inputs(x, wq, wk, wv, wo):
    bf = ml_dtypes.bfloat16
    x = np.asarray(x, dtype=np.float32)
    effs = {n: _bitlinear_eff(w) for n, w in
            (("wq", wq), ("wk", wk), ("wv", wv), ("wo", wo))}
    cmask = _causal_masks()
    xTs = [np.ascontiguousarray(x[b].T).astype(bf) for b in range(B)]
    shards = []
    for hg in range(2):
        rows = slice(hg * F_LOC, (hg + 1) * F_LOC)
        shards.append({
            "wqT": _tile_qkv(effs["wq"][rows, :]).astype(bf),
            "wkT": _tile_qkv(effs["wk"][rows, :]).astype(bf),
            "wvT": _tile_qkv(effs["wv"][rows, :]).astype(bf),
            "woT": _tile_wo(effs["wo"][:, rows]).astype(bf),
        })
    in_maps = []
    for c in range(N_CORES):
        b, hg = c // 2, c % 2
        m = {"xT": xTs[b], "cmask": cmask}
        m.update(shards[hg])
        in_maps.append(m)
    return in_maps


_NC_CACHE = {}


def _get_nc():
    if "nc" not in _NC_CACHE:
        _NC_CACHE["nc"] = build_bass()
    return _NC_CACHE["nc"]


def run(x, wq, wk, wv, wo, trace=False):
    nc = _get_nc()
    in_maps = _prep_inputs(x, wq, wk, wv, wo)
    res = bass_utils.run_bass_kernel_spmd(
        nc, in_maps, core_ids=list(range(N_CORES)), trace=trace)
    out = np.empty((B, T_FULL, D_MODEL), dtype=np.float32)
    for b in range(B):
        out[b] = (res.results[2 * b]["outT"]
                  + res.results[2 * b + 1]["outT"]).T
    return out, res


def kernel(x, wq, wk, wv, wo):
    out, _ = run(x, wq, wk, wv, wo)
    return out


# revision 42
# speedup vs baseline: 1.0091x; 1.0091x over previous
"""BitSelfAttention on 8 TRN2 NeuronCores.

Sharding: core c handles batch b = c//2 and head-group hg = c%2 (8 of 16 heads).
Each core computes its 8 heads' QKV projections + causal attention + its slice
of the o_proj GEMM, producing a partial output (transposed, [D, T], fp32).
Host: pre-quantizes BitLinear weights (ternary * gamma, exact in bf16),
pre-transposes operands into matmul-friendly layouts, and sums the two
head-group partials per batch at the end.

Device layouts (per core):
  xT   [D, T]  bf16 : x[b].T              (rhs for Q/K/V^T projections)
  wqT  [D, F]  bf16 : w_q_eff[hg-rows].T  (stationary tiles for Q^T proj)
  wkT  [D, F]  bf16
  wvT  [D, F]  bf16
  woT  [F, D]  bf16 : w_o_eff[:, hg-cols].T (stationary tiles for o_proj)
  cmask[4, 128, 512] bf16 : causal masks for the 4 diagonal offsets
  outT [D, T]  fp32 : partial output, transposed

Per head h: Q^T,K^T [dh=128, T] (dh-major), V^T transposed on the PE into
token-major V tiles. Attention computed as S^T = K^T_tile.T @ Q^T_block so
softmax rows land on the free axis; P^T = exp(S^T*scale) (ACT, PSUM->SBUF
bf16); key-tile partial row-sums accumulate in fp32 on the vector engine and
one all-ones stationary matmul per block reduces across partitions while
broadcasting the result to every partition (so normalization needs no
cross-partition broadcast); O^T = V_tile.T @ P^T accumulated over key tiles;
normalize with fast-reciprocal+multiply during PSUM eviction. o_proj consumes
O^T tiles directly as stationary operands, producing outT; its per-token-block
chains double as PE fill work zipped into the last head's attention, just as
each head's projection chains are zipped into the previous head's attention
(the attention inner loop is otherwise exp-latency-gated on the in-order PE).
"""

import math

import ml_dtypes
import numpy as np

import concourse.mybir as mybir
import concourse.tile as tile
from concourse import bacc
from concourse import bass_utils
from concourse.masks import make_identity

BF16 = mybir.dt.bfloat16
F32 = mybir.dt.float32

D_MODEL = 2048
N_HEAD = 16
D_HEAD = 128
B = 4
T_FULL = 2048
N_CORES = 8
F_LOC = D_MODEL // 2  # features per core (8 heads)


def build_bass(T=T_FULL, D=D_MODEL, F=F_LOC, debug=False):
    """Build the single-core program (SPMD across 8 cores via input data)."""
    P = 128
    KD = D // P      # contraction 128-tiles
    TT = T // P      # token 128-tiles
    TB = T // 512    # token 512-blocks
    H = F // P       # local heads
    MT = D // P      # output-dmodel 128-tiles
    KT_PER_B = 512 // P
    SCALE = 1.0 / math.sqrt(D_HEAD)

    nc = bacc.Bacc("TRN2", target_bir_lowering=False, debug=debug,
                   num_devices=N_CORES)
    xT_d = nc.dram_tensor("xT", [D, T], BF16, kind="ExternalInput").ap()
    # weights pre-tiled on host into the exact SBUF layouts (contiguous DMAs):
    #   wqT/wkT/wvT: [H, 128, KD*128] with [h, p, kd*128+f] = w_eff[h*128+f, kd*128+p]
    #   woT:         [MT, 128, H*128] with [m, p, h*128+j] = wo_eff[m*128+j, h*128+p]
    H_ = F // P
    MT_ = D // P
    KD_ = D // P
    wqT_d = nc.dram_tensor("wqT", [H_, P, KD_ * P], BF16,
                           kind="ExternalInput").ap()
    wkT_d = nc.dram_tensor("wkT", [H_, P, KD_ * P], BF16,
                           kind="ExternalInput").ap()
    wvT_d = nc.dram_tensor("wvT", [H_, P, KD_ * P], BF16,
                           kind="ExternalInput").ap()
    woT_d = nc.dram_tensor("woT", [MT_, P, H_ * P], BF16,
                           kind="ExternalInput").ap()
    cm_d = nc.dram_tensor("cmask", [4, P, 512], BF16, kind="ExternalInput").ap()
    out_d = nc.dram_tensor("outT", [D, T], F32, kind="ExternalOutput").ap()

    with tile.TileContext(nc) as tc:
        with (
            tc.tile_pool(name="big", bufs=1) as big,
            tc.tile_pool(name="work", bufs=2) as work,
            tc.tile_pool(name="psS", bufs=3, space="PSUM") as psS,
            tc.tile_pool(name="psO", bufs=2, space="PSUM") as psO,
            tc.tile_pool(name="psR", bufs=1, space="PSUM") as psR,
            tc.tile_pool(name="psP", bufs=2, space="PSUM") as psP,
        ):
            # ---- persistent inputs (head-0 weights first: first MMs need them)
            wvh0 = work.tile([P, KD, P], BF16, name="wvh0", tag="wvh")
            nc.sync.dma_start(out=wvh0.rearrange("p kd f -> p (kd f)"),
                              in_=wvT_d[0])
            ones = big.tile([P, P], BF16, name="ones_sb", tag="ones", bufs=1)
            nc.vector.memset(ones, 1.0)
            ident = big.tile([P, P], BF16, name="ident_sb", tag="ident", bufs=1)
            make_identity(nc, ident)
            xt = []
            for kd in range(KD):
                xti = big.tile([P, T], BF16, name=f"xt{kd}", tag="xt", bufs=KD)
                nc.sync.dma_start(out=xti, in_=xT_d[kd * P:(kd + 1) * P, :])
                xt.append(xti)
            cmask = big.tile([P, 4, 512], BF16, name="cmask_sb", tag="cmask",
                             bufs=1)
            for i in range(4):
                nc.sync.dma_start(out=cmask[:, i, :], in_=cm_d[i])
            ot = [big.tile([P, T], BF16, name=f"ot{h}", tag="ot", bufs=H)
                  for h in range(H)]

            # ---- per-head pipeline with cross-head fill interleaving.
            # The attention inner loop is ACT(exp)-gated by ~40ns/iter; we
            # pump one projection matmul of the NEXT head between attention
            # iterations so the (in-order) PE always has fill work.
            def load_head_weights(h, wvh=None):
                if wvh is None:
                    wvh = work.tile([P, KD, P], BF16, name=f"wvh{h}",
                                    tag="wvh")
                    nc.sync.dma_start(out=wvh.rearrange("p kd f -> p (kd f)"),
                                      in_=wvT_d[h])
                wqh = work.tile([P, KD, P], BF16, name=f"wqh{h}", tag="wqh")
                nc.sync.dma_start(out=wqh.rearrange("p kd f -> p (kd f)"),
                                  in_=wqT_d[h])
                wkh = work.tile([P, KD, P], BF16, name=f"wkh{h}", tag="wkh")
                nc.sync.dma_start(out=wkh.rearrange("p kd f -> p (kd f)"),
                                  in_=wkT_d[h])
                return wqh, wkh, wvh

            def load_qk_weights(h):
                wqh = work.tile([P, KD, P], BF16, name=f"wqh{h}", tag="wqh")
                nc.sync.dma_start(out=wqh.rearrange("p kd f -> p (kd f)"),
                                  in_=wqT_d[h])
                wkh = work.tile([P, KD, P], BF16, name=f"wkh{h}", tag="wkh")
                nc.sync.dma_start(out=wkh.rearrange("p kd f -> p (kd f)"),
                                  in_=wkT_d[h])
                return wqh, wkh

            def alloc_head_tiles(h):
                vT = work.tile([P, T], BF16, name=f"vT{h}", tag="vT")
                vh = work.tile([P, TT, P], BF16, name=f"vh{h}", tag="vh")
                qt_ = work.tile([P, T], BF16, name=f"qt{h}", tag="qt")
                kt_ = work.tile([P, T], BF16, name=f"kt{h}", tag="kt")
                return vT, vh, qt_, kt_

            def proj_fill_gen(ws, tiles):
                """V^T then Q^T then K^T projection chains, yielding after
                every matmul so the caller can interleave them."""
                wqh, wkh, wvh = ws
                vT, vh, qt_, kt_ = tiles
                for wh, dst in ((wvh, vT), (wqh, qt_), (wkh, kt_)):
                    for tb in range(TB):
                        ts_ = slice(tb * 512, (tb + 1) * 512)
                        ps = psP.tile([P, 512], F32, name="psfill", tag="psp")
                        for kd in range(KD):
                            nc.tensor.matmul(ps, lhsT=wh[:, kd, :],
                                             rhs=xt[kd][:, ts_],
                                             start=(kd == 0),
                                             stop=(kd == KD - 1))
                            yield
                        nc.vector.tensor_copy(out=dst[:, ts_], in_=ps)

            def pump(gen, n):
                for _ in range(n):
                    try:
                        next(gen)
                    except StopIteration:
                        return False
                return True

            def pump_n(gen, n):
                c = 0
                for _ in range(n):
                    try:
                        next(gen)
                        c += 1
                    except StopIteration:
                        break
                return c

            def oproj_nb_gen(nb):
                """o_proj chains for one token block (needs all heads' ot
                columns of that block only), yielding per matmul."""
                ns = slice(nb * 512, (nb + 1) * 512)
                for m in range(MT):
                    woh = work.tile([P, H, P], BF16, name=f"woh{nb}_{m}",
                                    tag="woh", bufs=4)
                    nc.sync.dma_start(out=woh.rearrange("p h f -> p (h f)"),
                                      in_=woT_d[m])
                    yield  # let attention matmuls cover the woh DMA latency
                    ps = psP.tile([P, 512], F32, name="psout", tag="psp")
                    for hh in range(H):
                        nc.tensor.matmul(ps, lhsT=woh[:, hh, :],
                                         rhs=ot[hh][:, ns],
                                         start=(hh == 0), stop=(hh == H - 1))
                        yield
                    stg = work.tile([P, 512], F32, name="ostage", tag="ostage",
                                    bufs=4)
                    nc.vector.tensor_copy(out=stg, in_=ps)
                    nc.sync.dma_start(out=out_d[m * P:(m + 1) * P, ns],
                                      in_=stg)

            # head-0 Q/K weights and head-1 weights load after xt (the V^T
            # chains consume xt first; the Q chains run ~4 chain-times later)
            ws_list = [None] * (H + 2)
            wqh0, wkh0 = load_qk_weights(0)
            ws_list[0] = (wqh0, wkh0, wvh0)
            if H > 1:
                ws_list[1] = load_head_weights(1)
            cur_tiles = alloc_head_tiles(0)
            g0 = proj_fill_gen(ws_list[0], cur_tiles)
            while pump(g0, 1):
                pass

            fills = []

            def pump_fills(n):
                while n > 0 and fills:
                    n -= pump_n(fills[0], n)
                    if n > 0:
                        fills.pop(0)

            for h in range(H):
                vT, vh, qt_, kt_ = cur_tiles
                # prefetch weights two heads ahead so fill matmuls never
                # wait on their DMA (a blocked fill stalls the in-order PE)
                if h + 2 < H:
                    ws_list[h + 2] = load_head_weights(h + 2)
                if h + 1 < H:
                    next_tiles = alloc_head_tiles(h + 1)
                    fills.append(proj_fill_gen(ws_list[h + 1], next_tiles))
                else:
                    next_tiles = None

                def emit_transpose(kt):
                    # lives in the psS pool: psP slots are held long by
                    # in-flight interleaved fill chains
                    pst = psS.tile([P, 512], BF16, name="pst", tag="pss")
                    nc.tensor.transpose(pst[:, 0:P],
                                        vT[:, kt * P:(kt + 1) * P], ident)
                    nc.vector.tensor_copy(out=vh[:, kt, :], in_=pst[:, 0:P])

                # causal attention, S^T layout (keys on partitions).
                # Diagonal tiles (kt = 4*qb+di) only contribute to query
                # columns >= 128*di of the block; narrow S/exp/O/R to the
                # live columns. Only the first 128 columns of a (narrowed)
                # diagonal tile are triangular; the rest are fully allowed.
                for qb in range(TB):
                    nkt = KT_PER_B * (qb + 1)
                    for kt in range(KT_PER_B * qb, nkt):
                        emit_transpose(kt)
                    psO_t = psO.tile([P, 512], F32, name="psodt", tag="pso")
                    racc = work.tile([P, 512], F32, name="racc", tag="racc")
                    for kt in range(nkt):
                        di = kt - KT_PER_B * qb
                        c0 = max(di, 0) * P  # first live query column
                        w = 512 - c0
                        qs = slice(qb * 512 + c0, (qb + 1) * 512)
                        psS_t = psS.tile([P, 512], F32, name="pssc", tag="pss")
                        nc.tensor.matmul(psS_t[:, :w],
                                         lhsT=kt_[:, kt * P:(kt + 1) * P],
                                         rhs=qt_[:, qs],
                                         start=True, stop=True)
                        pt = work.tile([P, 512], BF16, name="pexp", tag="pt",
                                       bufs=6)
                        nc.scalar.activation(
                            out=pt[:, :w], in_=psS_t[:, :w],
                            func=mybir.ActivationFunctionType.Exp, scale=SCALE)
                        if di >= 0:
                            nc.vector.tensor_mul(pt[:, :P], pt[:, :P],
                                                 cmask[:, 0, :P])
                        nc.tensor.matmul(psO_t[:, c0:], lhsT=vh[:, kt, :],
                                         rhs=pt[:, :w],
                                         start=(kt == 0), stop=(kt == nkt - 1),
                                         skip_group_check=True)
                        # fp32 running key-tile sum on DVE (hidden behind the
                        # exp pacing); one ones-matmul at the end reduces
                        # across partitions and broadcasts
                        if kt == 0:
                            nc.vector.tensor_copy(out=racc, in_=pt)
                        else:
                            nc.vector.tensor_add(racc[:, c0:], racc[:, c0:],
                                                 pt[:, :w])
                        pump_fills(1 + (kt & 1))
                    raccb = work.tile([P, 512], BF16, name="raccb", tag="raccb")
                    nc.vector.tensor_copy(out=raccb, in_=racc)
                    psR_t = psR.tile([P, 512], F32, name="psrow", tag="psr")
                    nc.tensor.matmul(psR_t, lhsT=ones, rhs=raccb,
                                     start=True, stop=True)
                    rec = work.tile([P, 512], F32, name="rec", tag="rec")
                    nc.vector.reciprocal_approx_fast(out=rec, in_=psR_t)
                    nc.vector.tensor_mul(ot[h][:, qb * 512:(qb + 1) * 512],
                                         psO_t, rec)
                    if h == H - 1:
                        # this token block's ot columns are now complete for
                        # every head: its o_proj chains become fill work
                        fills.append(oproj_nb_gen(qb))
                    pump_fills(4)
                if h < H - 1:
                    # finish next head's projections before its attention
                    while fills:
                        pump_fills(64)
                cur_tiles = next_tiles
            # drain remaining o_proj work
            while fills:
                pump_fills(64)

    nc.compile()
    return nc


def _bitlinear_eff(w):
    """Forward-effective BitLinear weight: clip(round(w/gamma),-1,1)*gamma."""
    w = np.asarray(w, dtype=np.float32)
    gamma = max(np.float32(np.abs(w).mean()), np.float32(1e-5))
    q = np.clip(np.round(w / gamma), -1.0, 1.0).astype(np.float32)
    return q * gamma


def _causal_masks():
    k = np.arange(128)[:, None]
    q = np.arange(512)[None, :]
    m = np.stack([(k <= q - 128 * i) for i in range(4)]).astype(np.float32)
    return m.astype(ml_dtypes.bfloat16)


def _tile_qkv(w_shard):
    """[F, D] -> [H, 128, KD*128]: [h, p, kd*128+f] = w_shard[h*128+f, kd*128+p]."""
    Fs, Ds = w_shard.shape
    a = w_shard.reshape(Fs // 128, 128, Ds // 128, 128)  # [h, f, kd, p]
    a = a.transpose(0, 3, 2, 1).reshape(Fs // 128, 128, Ds)
    return np.ascontiguousarray(a)


def _tile_wo(wo_shard):
    """[D, F] -> [MT, 128, H*128]: [m, p, h*128+j] = wo_shard[m*128+j, h*128+p]."""
    Ds, Fs = wo_shard.shape
    a = wo_shard.reshape(Ds // 128, 128, Fs // 128, 128)  # [m, j, h, p]
    a = a.transpose(0, 3, 2, 1).reshape(Ds // 128, 128, Fs)
    return np.ascontiguousarray(a)


def _prep_inputs(x, wq, wk, wv, wo):
    bf = ml_dtypes.bfloat16
    x = np.asarray(x, dtype=np.float32)
    effs = {n: _bitlinear_eff(w) for n, w in
            (("wq", wq), ("wk", wk), ("wv", wv), ("wo", wo))}
    cmask = _causal_masks()
    xTs = [np.ascontiguousarray(x[b].T).astype(bf) for b in range(B)]
    shards = []
    for hg in range(2):
        rows = slice(hg * F_LOC, (hg + 1) * F_LOC)
        shards.append({
            "wqT": _tile_qkv(effs["wq"][rows, :]).astype(bf),
            "wkT": _tile_qkv(effs["wk"][rows, :]).astype(bf),
            "wvT": _tile_qkv(effs["wv"][rows, :]).astype(bf),
            "woT": _tile_wo(effs["wo"][:, rows]).astype(bf),
        })
    in_maps = []
    for c in range(N_CORES):
        b, hg = c // 2, c % 2
        m = {"xT": xTs[b], "cmask": cmask}
        m.update(shards[hg])
        in_maps.append(m)
    return in_maps


_NC_CACHE = {}


def _get_nc():
    if "nc" not in _NC_CACHE:
        _NC_CACHE["nc"] = build_bass()
    return _NC_CACHE["nc"]


def run(x, wq, wk, wv, wo, trace=False):
    nc = _get_nc()
    in_maps = _prep_inputs(x, wq, wk, wv, wo)
    res = bass_utils.run_bass_kernel_spmd(
        nc, in_maps, core_ids=list(range(N_CORES)), trace=trace)
    out = np.empty((B, T_FULL, D_MODEL), dtype=np.float32)
    for b in range(B):
        out[b] = (res.results[2 * b]["outT"]
                  + res.results[2 * b + 1]["outT"]).T
    return out, res


def kernel(x, wq, wk, wv, wo):
    out, _ = run(x, wq, wk, wv, wo)
    return out
